# revision 5
# baseline (speedup 1.0000x reference)
"""DTW kernel for nn_DTW_71236327571899.

Single (y, y_hat) pair, both (4096, 16) fp32; output is the scalar DTW
cost C[4095, 4095] of the standard recurrence
    C[i,j] = D[i,j] + min(C[i-1,j], C[i,j-1], C[i-1,j-1]),
with D the per-pair mean squared distance.

Single fused pass (this box has 1 CPU core and no optimized BLAS, so
neither the 67MB distance matrix nor the 134MB skewed-diagonal matrix
is ever materialized). Per antidiagonal k the DP runs only over the
valid band i in [k-4095, k] clamped (halves total cells vs a
full-width scan), and the distance D[i, k-i] is computed inline.
The trick making it SIMD-friendly: with yT (c-major y) and yhR
(c-major y_hat, reversed along time), the diagonal's distance operand
  y_hat[k-i, c] == yhR[c, i + (H-1-k)]
is CONTIGUOUS in i, so the whole band loop (16 FMAs + 2 mins + add per
cell) vectorizes with unit stride and no bounds checks.
"""

import numpy as np

H = 4096
C = 16


def _get_jit():
    global _JIT
    try:
        return _JIT
    except NameError:
        pass
    import numba

    @numba.njit(cache=True, fastmath=True)
    def _dtw(yT, yhR):
        INF = np.float32(1e30)
        nk = 2 * H - 1
        inv = np.float32(1.0 / C)

        two = np.full(H + 1, INF, np.float32)
        one = np.full(H + 1, INF, np.float32)
        nxt = np.full(H + 1, INF, np.float32)

        # k = 0: E[0,0] = D[0,0];  k = 1: E[1,0] = D[0,1]+E00, E[1,1] = D[1,0]+E00
        d00 = np.float32(0.0)
        d01 = np.float32(0.0)
        d10 = np.float32(0.0)
        for c in range(C):
            e0 = yT[c, 0] - yhR[c, H - 1]
            e1 = yT[c, 0] - yhR[c, H - 2]
            e2 = yT[c, 1] - yhR[c, H - 1]
            d00 += e0 * e0
            d01 += e1 * e1
            d10 += e2 * e2
        two[1] = d00 * inv
        one[1] = d01 * inv + two[1]
        one[2] = d10 * inv + two[1]

        for k in range(2, nk):
            ilo = 0 if k <= H - 1 else k - (H - 1)
            ihi = k if k <= H - 1 else H - 1
            off = H - 1 - k
            for i in range(ilo, ihi + 1):
                s = np.float32(0.0)
                for c in range(C):
                    e = yT[c, i] - yhR[c, i + off]
                    s += e * e
                a = two[i]
                b = one[i]
                cc = one[i + 1]
                m = a if a < b else b
                m = m if m < cc else cc
                nxt[i + 1] = m + s * inv
            t = two
            two = one
            one = nxt
            nxt = t
        return one[H]

    _JIT = _dtw
    return _JIT


def kernel(y, y_hat):
    y = np.asarray(y, dtype=np.float32)
    y_hat = np.asarray(y_hat, dtype=np.float32)
    yT = np.ascontiguousarray(y.T)
    yhR = np.ascontiguousarray(y_hat[::-1].T)
    try:
        fn = _get_jit()
        return np.float32(fn(yT, yhR))
    except Exception:
        return _kernel_fallback(y, y_hat)


def _kernel_fallback(y, y_hat):
    # pure-numpy fallback (identical math, no numba)
    G = y @ y_hat.T
    a = np.sum(y * y, axis=1, dtype=np.float32)
    b = np.sum(y_hat * y_hat, axis=1, dtype=np.float32)
    D = ((a[:, None] + b[None, :] - 2.0 * G) / np.float32(y.shape[1])).astype(
        np.float32
    )
    np.maximum(D, 0.0, out=D)
    INF = np.float32(np.inf)
    nk = 2 * H - 1
    flat = np.full(H * (H + 1) + 8, INF, np.float32)
    flat[: H * (H + 1)].reshape(H, H + 1)[:, :H] = D
    from numpy.lib.stride_tricks import as_strided

    M = as_strided(flat, shape=(nk, H), strides=(4, 4 * H))
    two = np.full(H + 1, INF, np.float32)
    one = np.full(H + 1, INF, np.float32)
    nxt = np.empty(H + 1, np.float32)
    nxt[0] = INF
    best = np.empty(H, np.float32)
    two[1:] = M[0]
    np.add(M[1], M[0, 0], out=one[1:])
    for k in range(2, nk):
        np.minimum(two[:-1], one[:-1], out=best)
        np.minimum(best, one[1:], out=best)
        np.add(best, M[k], out=nxt[1:])
        two, one, nxt = one, nxt, two
    return np.float32(one[H])
